# revision 1
# baseline (speedup 1.0000x reference)
"""Trainium2 Bass kernel for nn_AttentionBlock (B=4, S=2048, H=1024, NH=16).

Sharding: 8 cores = 4 batches x 2 head-groups (8 heads each).
Each core computes, for its (batch b, head-group g):
    partial_out[q, :] = attn(x_b)[:, heads 8g..8g+8] @ Wo[512g:512g+512, :]
Host sums the two partials per batch, adds residual x and bo.

Device-side pipeline (per core):
    prologue: DMA xT/W; V = x Wv + bv (mask folded, ones col for Z); Q^T/K^T pair 0
    per head-pair j: per q-tile: loop k-blocks:
        S^T = K_l Q_l^T (heads 2j/2j+1 row-tiled in the PE array)
        exp(S^T/8) on ScalarE (PSUM->SBUF, scale fused)
        AV accumulate [65, q]: rows 0..63 numer^T, row 64 = Z  (lagged one k-block)
        background work interleaved: Q^T/K^T of pair j+1, Wo of finished q ranges
    epilogue per (head, qtile): 1/Z (recip) -> C=1 broadcast matmul -> DVE mul
    Wo: out_partial[q,:] = attn^T @ Wo_g
"""

import os

import numpy as np
import ml_dtypes

B, S, H, NH = 4, 2048, 1024, 16
HD = H // NH          # 64
G = 2                 # head groups (tensor-parallel factor)
HPC = NH // G         # heads per core = 8
DG = HPC * HD         # 512, d-width per core
NCORES = 8

P = 128               # partitions
FQ = 512              # q tile (matmul free dim)
NQT = S // FQ         # 4 q tiles
NKB = S // P          # 16 k blocks
NHC = H // P          # 8 h chunks
NDB = DG // P         # 4 d blocks (= head pairs)
NQB = S // P          # 16 q row-blocks (for Wo)
SCALE = 1.0 / np.sqrt(HD)

_CACHE = {}


def _build(reps=1, loop=None, phases="all"):
    """Build the program. reps>1 unrolls the whole computation N times in one
    NEFF (same buffers); loop=N wraps the body in a hardware For loop executing
    it N times; phases selects a prefix of the pipeline ("dma"|"qkv"|"attn"|"all")
    for bisection benchmarks. The graded path uses reps=1, loop=None, "all"."""
    import concourse.bacc as bacc
    import concourse.mybir as mybir
    from concourse import tile

    dt = mybir.dt
    f32, bf16 = dt.float32, dt.bfloat16
    AF = mybir.ActivationFunctionType
    OP = mybir.AluOpType

    nc = bacc.Bacc("TRN2", target_bir_lowering=False, debug=False)

    xT = nc.dram_tensor("xT", [H, S], bf16, kind="ExternalInput")
    wq = nc.dram_tensor("wq", [H, DG], bf16, kind="ExternalInput")
    wk = nc.dram_tensor("wk", [H, DG], bf16, kind="ExternalInput")
    wv = nc.dram_tensor("wv", [H, DG], bf16, kind="ExternalInput")
    wo = nc.dram_tensor("wo", [DG, H], bf16, kind="ExternalInput")
    bqd = nc.dram_tensor("bqd", [P, NDB], f32, kind="ExternalInput")
    bkd = nc.dram_tensor("bkd", [P, NDB], f32, kind="ExternalInput")
    bvd = nc.dram_tensor("bvd", [1, DG], bf16, kind="ExternalInput")
    maskd = nc.dram_tensor("maskd", [P, NKB], f32, kind="ExternalInput")
    outd = nc.dram_tensor("out", [S, H], f32, kind="ExternalOutput")

    with tile.TileContext(nc) as tc:
        with (
            tc.tile_pool(name="const", bufs=1) as cpool,
            tc.tile_pool(name="big", bufs=1) as bpool,
            tc.tile_pool(name="work", bufs=1) as wpool,
            tc.tile_pool(name="ps", bufs=1, space="PSUM") as psp,
        ):
            ones_bf = cpool.tile([1, P], bf16, name="ones_bf", tag="ones_bf")
            nc.vector.memset(ones_bf[:, :], 1.0)
            bq_sb = cpool.tile([P, NDB], f32, name="bq_sb", tag="bq_sb")
            bk_sb = cpool.tile([P, NDB], f32, name="bk_sb", tag="bk_sb")
            bv_sb = cpool.tile([1, DG], bf16, name="bv_sb", tag="bv_sb")
            mask_sb = cpool.tile([P, NKB], f32, name="mask_sb", tag="mask_sb")

            import contextlib
            loop_cm = tc.For_i(0, loop, 1) if loop else contextlib.nullcontext()
            with loop_cm:
                for _rep in range(reps):
                    # x^T in SBUF: [p, hc, q] with h = hc*128 + p
                    xt_sb = bpool.tile([P, NHC, S], bf16, name="xt_sb", tag="xt_sb")
                    wq_sb = bpool.tile([P, NHC, DG], bf16, name="wq_sb", tag="wq_sb")
                    wk_sb = bpool.tile([P, NHC, DG], bf16, name="wk_sb", tag="wk_sb")
                    wv_sb = bpool.tile([P, NHC, DG], bf16, name="wv_sb", tag="wv_sb")
                    wo_sb = bpool.tile([P, NDB, H], bf16, name="wo_sb", tag="wo_sb")
                    nc.sync.dma_start(xt_sb[:, :, :], xT.rearrange("(c p) q -> p c q", p=P))
                    nc.sync.dma_start(wv_sb[:, :, :], wv.rearrange("(c p) d -> p c d", p=P))
                    nc.sync.dma_start(wq_sb[:, :, :], wq.rearrange("(c p) d -> p c d", p=P))
                    nc.sync.dma_start(wk_sb[:, :, :], wk.rearrange("(c p) d -> p c d", p=P))
                    nc.sync.dma_start(wo_sb[:, :, :], wo.rearrange("(c p) d -> p c d", p=P))
                    nc.sync.dma_start(bq_sb[:, :], bqd[:, :])
                    nc.sync.dma_start(bk_sb[:, :], bkd[:, :])
                    nc.sync.dma_start(bv_sb[:, :], bvd[:, :])
                    nc.sync.dma_start(mask_sb[:, :], maskd[:, :])

                    qt_sb = [bpool.tile([P, S], bf16, name=f"qt{j}", tag=f"qt{j}") for j in range(NDB)]
                    kt_sb = [bpool.tile([P, S], bf16, name=f"kt{j}", tag=f"kt{j}") for j in range(NDB)]
                    v_sb = [bpool.tile([P, HPC * (HD + 1)], bf16, name=f"v{k}", tag=f"v{k}")
                            for k in range(NKB)]
                    at_sb = [bpool.tile([P, S], bf16, name=f"at{j}", tag=f"at{j}") for j in range(NDB)]

                    # ---- background work generators (each item issues a small MM group).
                    # Emission interleaves two accumulation groups so consecutive
                    # matmuls target different PSUM banks (same-bank back-to-back
                    # accumulate stalls the PE drain). ----
                    def qkt_group(j, t, which):
                        w_sb, b_sb, dst, tg = ((wq_sb, bq_sb, qt_sb, "bg0") if which == "q"
                                               else (wk_sb, bk_sb, kt_sb, "bg1"))

                        def run():
                            pg = psp.tile([P, FQ], f32, name=f"p{which}_{j}_{t}", tag=tg, bufs=1)
                            for c in range(NHC):
                                nc.tensor.matmul(
                                    pg[:, :],
                                    lhsT=w_sb[:, c, j * P:(j + 1) * P],
                                    rhs=xt_sb[:, c, t * FQ:(t + 1) * FQ],
                                    start=(c == 0), stop=(c == NHC - 1),
                                )
                            nc.vector.tensor_scalar(
                                out=dst[j][:, t * FQ:(t + 1) * FQ],
                                in0=pg[:, :], scalar1=b_sb[:, j:j + 1], scalar2=None,
                                op0=OP.add,
                            )
                        return run

                    def v_pair_group(kb0):
                        def run():
                            pvs = [psp.tile([P, DG], f32, name=f"pv_{kb0}_{i}",
                                            tag=f"bg{i}", bufs=1) for i in range(2)]
                            for i in range(2):
                                for c in range(NHC):
                                    nc.tensor.matmul(
                                        pvs[i][:, :],
                                        lhsT=xt_sb[:, c, (kb0 + i) * P:(kb0 + i + 1) * P],
                                        rhs=wv_sb[:, c, :],
                                        start=(c == 0), stop=False,
                                    )
                                nc.tensor.matmul(
                                    pvs[i][:, :], lhsT=ones_bf[0:1, 0:P], rhs=bv_sb[0:1, :],
                                    start=False, stop=True,
                                )
                            for i in range(2):
                                kb = kb0 + i
                                vt = v_sb[kb].rearrange("p (l c) -> p l c", c=HD + 1)
                                nc.vector.memset(vt[:, :, HD:HD + 1], 1.0)
                                nc.vector.tensor_scalar(
                                    out=vt[:, :, 0:HD],
                                    in0=pvs[i].rearrange("p (l d) -> p l d", d=HD),
                                    scalar1=mask_sb[:, kb:kb + 1], scalar2=None,
                                    op0=OP.mult,
                                )
                        return run

                    def wo_group(qb):
                        def run():
                            po = [psp.tile([P, FQ], f32, name=f"po{n}_{qb}", tag=f"bg{n}", bufs=1)
                                  for n in range(2)]
                            for n in range(2):
                                for j in range(NDB):
                                    nc.tensor.matmul(
                                        po[n][:, :],
                                        lhsT=at_sb[j][:, qb * P:(qb + 1) * P],
                                        rhs=wo_sb[:, j, n * FQ:(n + 1) * FQ],
                                        start=(j == 0), stop=(j == NDB - 1),
                                    )
                            ob = wpool.tile([P, H], f32, name=f"ob_{qb}", tag="ob", bufs=3)
                            for n in range(2):
                                nc.vector.tensor_copy(ob[:, n * FQ:(n + 1) * FQ], po[n][:, :])
                            nc.sync.dma_start(outd[qb * P:(qb + 1) * P, :], ob[:, :])
                        return run

                    # ---- prologue: V (all k-blocks) + Q^T/K^T for pair 0 ----
                    if phases == "dma":
                        continue
                    for kb0 in range(0, NKB, 2):
                        v_pair_group(kb0)()
                    for t in range(NQT):
                        qkt_group(0, t, "q")()
                        qkt_group(0, t, "k")()
                    if phases == "qkv":
                        for j in range(1, NDB):
                            for t in range(NQT):
                                qkt_group(j, t, "q")()
                                qkt_group(j, t, "k")()
                        continue

                    # ---- attention with interleaved background work ----
                    background = []   # queue of callables issued one per k-block iteration
                    for j in range(NDB):
                        if j + 1 < NDB:
                            for t in range(NQT):
                                background.append(qkt_group(j + 1, t, "q"))
                                background.append(qkt_group(j + 1, t, "k"))
                        for qt in range(NQT):
                            if phases not in ("attn", "noav", "noexp") and j == NDB - 1 and qt > 0:
                                # Wo for q row-blocks of the previous q-tile (all pairs done)
                                for qb in range((qt - 1) * (FQ // P), qt * (FQ // P)):
                                    background.append(wo_group(qb))
                            av = [psp.tile([HD + 1, FQ], f32, name=f"av{hh}_{j}_{qt}",
                                           tag=f"av{hh}") for hh in range(2)]
                            es = []
                            for kbp in range(NKB // 2):   # k-block pairs
                                ps_pair = [psp.tile([P, 2 * FQ], f32, name=f"s{hh}_{j}_{qt}_{kbp}",
                                                    tag=f"s{hh}", bufs=1) for hh in range(2)]
                                for sub in range(2):
                                    kb = 2 * kbp + sub
                                    for hh, base in ((0, 0), (1, HD)):
                                        nc.tensor.matmul(
                                            ps_pair[hh][:, sub * FQ:(sub + 1) * FQ],
                                            lhsT=kt_sb[j][base:base + HD, kb * P:(kb + 1) * P],
                                            rhs=qt_sb[j][base:base + HD, qt * FQ:(qt + 1) * FQ],
                                            start=True, stop=True,
                                        )
                                e_t = []
                                if phases != "noexp":
                                    for hh in range(2):
                                        e = wpool.tile([P, 2 * FQ], bf16, name=f"e{hh}_{j}_{qt}_{kbp}",
                                                       tag="e", bufs=20)
                                        nc.scalar.activation(e[:, :], ps_pair[hh][:, :], AF.Exp,
                                                             scale=float(SCALE))
                                        e_t.append(e)
                                else:
                                    e = wpool.tile([P, 2 * FQ], bf16, name=f"ecp_{j}_{qt}_{kbp}",
                                                   tag="e", bufs=20)
                                    nc.vector.tensor_copy(e[:, 0:FQ], ps_pair[0][:, 0:FQ])
                                    nc.vector.tensor_copy(e[:, FQ:2 * FQ], ps_pair[1][:, 0:FQ])
                                es.append(e_t)
                                if background and (kbp % 2 == 1):
                                    background.pop(0)()
                            if phases not in ("noav", "noexp"):
                                # AV burst: clean alternating-bank accumulation run
                                for kbp in range(NKB // 2):
                                    for sub in range(2):
                                        kb = 2 * kbp + sub
                                        for hh in range(2):
                                            l = 2 * j + hh
                                            nc.tensor.matmul(
                                                av[hh][:, :],
                                                lhsT=v_sb[kb][:, l * (HD + 1):(l + 1) * (HD + 1)],
                                                rhs=es[kbp][hh][:, sub * FQ:(sub + 1) * FQ],
                                                start=(kb == 0), stop=(kb == NKB - 1),
                                            )
                            # epilogue: divide by Z, write attn^T rows for both heads
                            for hh in range(2):
                                if phases in ("noav", "noexp"):
                                    break
                                a = av[hh]
                                zr = wpool.tile([1, FQ], bf16, name=f"zr_{j}_{qt}_{hh}",
                                                tag="zr", bufs=2)
                                with nc.allow_low_precision(reason="1/Z in bf16 is within absmax budget"):
                                    nc.vector.reciprocal(zr[0:1, :], a[HD:HD + 1, :])
                                pbc = psp.tile([HD, FQ], f32, name=f"bc_{j}_{qt}_{hh}",
                                               tag=f"s{hh}", bufs=1)
                                nc.tensor.matmul(
                                    pbc[:, :], lhsT=ones_bf[0:1, 0:HD], rhs=zr[0:1, :],
                                    start=True, stop=True,
                                )
                                nm = wpool.tile([HD, FQ], f32, name=f"nm_{j}_{qt}_{hh}",
                                                tag="nm", bufs=2)
                                nc.vector.tensor_copy(nm[:, :], a[0:HD, :])
                                nc.vector.tensor_mul(
                                    at_sb[j][hh * HD:(hh + 1) * HD, qt * FQ:(qt + 1) * FQ],
                                    nm[:, :], pbc[:, :],
                                )
                    # remaining background (last q-tile's Wo) + final q-tile row-blocks
                    for fn in background:
                        fn()
                    if phases not in ("attn", "noav", "noexp"):
                        for qb in range((NQT - 1) * (FQ // P), NQT * (FQ // P)):
                            wo_group(qb)()

    nc.compile()
    return nc


def _issue_av(nc, prev, av, v_sb, j):
    e_t, kbp = prev
    for sub in range(2):
        kb = 2 * kbp + sub
        for hh in range(2):
            l = 2 * j + hh
            nc.tensor.matmul(
                av[hh][:, :],
                lhsT=v_sb[kb][:, l * (HD + 1):(l + 1) * (HD + 1)],
                rhs=e_t[hh][:, sub * FQ:(sub + 1) * FQ],
                start=(kb == 0), stop=(kb == NKB - 1),
            )


def _shard_inputs(inputs, radial_mask, Wq, bq, Wk, bk, Wv, bv, Wo):
    bf16 = ml_dtypes.bfloat16
    in_maps = []
    for c in range(NCORES):
        b, g = c // G, c % G
        sl = slice(DG * g, DG * (g + 1))
        in_maps.append({
            "xT": np.ascontiguousarray(inputs[b].T).astype(bf16),
            "wq": np.ascontiguousarray(Wq[:, sl]).astype(bf16),
            "wk": np.ascontiguousarray(Wk[:, sl]).astype(bf16),
            "wv": np.ascontiguousarray(Wv[:, sl]).astype(bf16),
            "wo": np.ascontiguousarray(Wo[sl, :]).astype(bf16),
            "bqd": np.ascontiguousarray(bq[sl].reshape(NDB, P).T).astype(np.float32),
            "bkd": np.ascontiguousarray(bk[sl].reshape(NDB, P).T).astype(np.float32),
            "bvd": np.ascontiguousarray(bv[sl].reshape(1, DG)).astype(bf16),
            "maskd": np.ascontiguousarray(radial_mask[b].reshape(NKB, P).T).astype(np.float32),
        })
    return in_maps


def kernel(**inputs):
    from concourse.bass_utils import run_bass_kernel_spmd

    if "nc" not in _CACHE:
        _CACHE["nc"] = _build()
    nc = _CACHE["nc"]

    x = np.asarray(inputs["inputs"], np.float32)
    in_maps = _shard_inputs(
        x, np.asarray(inputs["radial_mask"], np.float32),
        np.asarray(inputs["Wq"], np.float32), np.asarray(inputs["bq"], np.float32),
        np.asarray(inputs["Wk"], np.float32), np.asarray(inputs["bk"], np.float32),
        np.asarray(inputs["Wv"], np.float32), np.asarray(inputs["bv"], np.float32),
        np.asarray(inputs["Wo"], np.float32),
    )

    trace = bool(int(os.environ.get("KERNEL_TRACE", "0")))
    res = run_bass_kernel_spmd(nc, in_maps, core_ids=list(range(NCORES)), trace=trace)
    _CACHE["last_result"] = res

    bo = np.asarray(inputs["bo"], np.float32)
    out = np.empty((B, S, H), np.float32)
    for b in range(B):
        out[b] = res.results[G * b]["out"] + res.results[G * b + 1]["out"] + x[b] + bo
    return out



# revision 4
# speedup vs baseline: 1.2632x; 1.2632x over previous
"""Trainium2 Bass kernel for nn_AttentionBlock (B=4, S=2048, H=1024, NH=16).

Sharding: 8 cores = 4 batches x 2 head-groups (8 heads each).
Each core computes, for its (batch b, head-group g):
    partial_out[q, :] = attn(x_b)[:, heads 8g..8g+8] @ Wo[512g:512g+512, :]
Host sums the two partials per batch, adds residual x and bo.

Device pipeline (per core), designed so ScalarE (exp stream, the floor at
~285us) and TensorE overlap near-fully and the PE never idles (HAM warm):

  prologue: DMA consts; wk/xT/wq chunk-interleaved; K(j=0), Q(j=0)
  per head-pair j (4), per q-tile qt (4):
    per k-block kb (16 slots):
      scores: 2 row-tiled concurrent MMs (head0 rows 0-63, head1 rows 64-127)
              into a double-buffered [128, 1024] f32 psum pair (2+2 banks)
      exp:    one ACTIVATE [128, 1024] psum->sbuf bf16 (scale=1/8 fused)
      AV:     lagged one slot; 2 MMs accumulate [65, q] (V row 64 = ones -> Z)
      bg:     one background group every other slot (V proj for j0/qt0,
              Q/K of pair j+1, Wo of finished q rows) in 2 spare psum banks
    epilogue: Z reciprocal via reciprocal_approx_fast (not the 8cyc/elem
              iterative divide), ones-broadcast MM, DVE mul -> at^T bf16
  Wo: out[qb,:] = attn^T @ Wo_g per 128-row block, f32 out
"""

import os

import numpy as np
import ml_dtypes

B, S, H, NH = 4, 2048, 1024, 16
HD = H // NH          # 64
G = 2                 # head groups (tensor-parallel factor)
HPC = NH // G         # heads per core = 8
DG = HPC * HD         # 512, d-width per core
NCORES = 8

P = 128               # partitions
FQ = 512              # q tile (matmul free dim)
NQT = S // FQ         # 4 q tiles
NKB = S // P          # 16 k blocks
NHC = H // P          # 8 h chunks
NDB = DG // P         # 4 d blocks (= head pairs)
NQB = S // P          # 16 q row-blocks (for Wo)
SCALE = 1.0 / np.sqrt(HD)

_CACHE = {}


def _build():
    import concourse.bacc as bacc
    import concourse.mybir as mybir
    from concourse import tile

    dt = mybir.dt
    f32, bf16 = dt.float32, dt.bfloat16
    AF = mybir.ActivationFunctionType
    OP = mybir.AluOpType

    nc = bacc.Bacc("TRN2", target_bir_lowering=False, debug=False)

    xT = nc.dram_tensor("xT", [H, S], bf16, kind="ExternalInput")
    wq = nc.dram_tensor("wq", [H, DG], bf16, kind="ExternalInput")
    wk = nc.dram_tensor("wk", [H, DG], bf16, kind="ExternalInput")
    wv = nc.dram_tensor("wv", [H, DG], bf16, kind="ExternalInput")
    wo = nc.dram_tensor("wo", [DG, H], bf16, kind="ExternalInput")
    bqd = nc.dram_tensor("bqd", [P, NDB], f32, kind="ExternalInput")
    bkd = nc.dram_tensor("bkd", [P, NDB], f32, kind="ExternalInput")
    bvd = nc.dram_tensor("bvd", [1, DG], bf16, kind="ExternalInput")
    maskd = nc.dram_tensor("maskd", [P, NKB], f32, kind="ExternalInput")
    outd = nc.dram_tensor("out", [S, H], f32, kind="ExternalOutput")

    with tile.TileContext(nc) as tc:
        with (
            tc.tile_pool(name="const", bufs=1) as cpool,
            tc.tile_pool(name="big", bufs=1) as bpool,
            tc.tile_pool(name="work", bufs=1) as wpool,
            tc.tile_pool(name="ps", bufs=1, space="PSUM") as psp,
        ):
            ones_bf = cpool.tile([1, P], bf16, name="ones_bf", tag="ones_bf")
            nc.vector.memset(ones_bf[:, :], 1.0)
            bq_sb = cpool.tile([P, NDB], f32, name="bq_sb", tag="bq_sb")
            bk_sb = cpool.tile([P, NDB], f32, name="bk_sb", tag="bk_sb")
            bv_sb = cpool.tile([1, DG], bf16, name="bv_sb", tag="bv_sb")
            mask_sb = cpool.tile([P, NKB], f32, name="mask_sb", tag="mask_sb")
            nc.sync.dma_start(bq_sb[:, :], bqd[:, :])
            nc.sync.dma_start(bk_sb[:, :], bkd[:, :])
            nc.sync.dma_start(bv_sb[:, :], bvd[:, :])
            nc.sync.dma_start(mask_sb[:, :], maskd[:, :])

            # x^T in SBUF, one tile per 128-row h-chunk for precise DMA deps
            xt_sb = [bpool.tile([P, S], bf16, name=f"xt{c}", tag=f"xt{c}")
                     for c in range(NHC)]
            wq_sb = bpool.tile([P, NHC, DG], bf16, name="wq_sb", tag="wq_sb")
            wk_sb = bpool.tile([P, NHC, DG], bf16, name="wk_sb", tag="wk_sb")
            wv_sb = bpool.tile([P, NHC, DG], bf16, name="wv_sb", tag="wv_sb")
            wo_sb = bpool.tile([P, NDB, H], bf16, name="wo_sb", tag="wo_sb")
            wkr = wk.rearrange("(c p) d -> p c d", p=P)
            wqr = wq.rearrange("(c p) d -> p c d", p=P)
            # chunk-interleave wk/x so prologue K matmuls start early
            for c in range(NHC):
                nc.sync.dma_start(wk_sb[:, c, :], wkr[:, c, :])
                nc.sync.dma_start(xt_sb[c][:, :], xT[c * P:(c + 1) * P, :])
            for c in range(NHC):
                nc.sync.dma_start(wq_sb[:, c, :], wqr[:, c, :])
            nc.sync.dma_start(wv_sb[:, :, :], wv.rearrange("(c p) d -> p c d", p=P))
            nc.sync.dma_start(wo_sb[:, :, :], wo.rearrange("(c p) d -> p c d", p=P))

            qt_sb = [bpool.tile([P, S], bf16, name=f"qt{j}", tag=f"qt{j}") for j in range(NDB)]
            kt_sb = [bpool.tile([P, S], bf16, name=f"kt{j}", tag=f"kt{j}") for j in range(NDB)]
            v_sb = [bpool.tile([P, HPC * (HD + 1)], bf16, name=f"v{k}", tag=f"v{k}")
                    for k in range(NKB)]
            at_sb = [bpool.tile([P, S], bf16, name=f"at{j}", tag=f"at{j}") for j in range(NDB)]

            # ---- background group generators (each issues one MM group into
            # a spare psum bank bg0/bg1 and evacuates via DVE) ----
            def qkt_group(j, t, which):
                w_sb, b_sb, dst, tg = ((wq_sb, bq_sb, qt_sb, "bg0") if which == "q"
                                       else (wk_sb, bk_sb, kt_sb, "bg1"))

                def run():
                    pg = psp.tile([P, FQ], f32, name=f"p{which}_{j}_{t}", tag=tg, bufs=1)
                    for c in range(NHC):
                        nc.tensor.matmul(
                            pg[:, :],
                            lhsT=w_sb[:, c, j * P:(j + 1) * P],
                            rhs=xt_sb[c][:, t * FQ:(t + 1) * FQ],
                            start=(c == 0), stop=(c == NHC - 1),
                        )
                    nc.vector.tensor_scalar(
                        out=dst[j][:, t * FQ:(t + 1) * FQ],
                        in0=pg[:, :], scalar1=b_sb[:, j:j + 1], scalar2=None,
                        op0=OP.add,
                    )
                return run

            def v_group(kb):
                def run():
                    pv = psp.tile([P, DG], f32, name=f"pv_{kb}", tag=f"bg{kb % 2}", bufs=1)
                    for c in range(NHC):
                        nc.tensor.matmul(
                            pv[:, :],
                            lhsT=xt_sb[c][:, kb * P:(kb + 1) * P],
                            rhs=wv_sb[:, c, :],
                            start=(c == 0), stop=False,
                        )
                    nc.tensor.matmul(
                        pv[:, :], lhsT=ones_bf[0:1, 0:P], rhs=bv_sb[0:1, :],
                        start=False, stop=True,
                    )
                    vt = v_sb[kb].rearrange("p (l c) -> p l c", c=HD + 1)
                    nc.vector.memset(vt[:, :, HD:HD + 1], 1.0)
                    nc.vector.tensor_scalar(
                        out=vt[:, :, 0:HD],
                        in0=pv.rearrange("p (l d) -> p l d", d=HD),
                        scalar1=mask_sb[:, kb:kb + 1], scalar2=None,
                        op0=OP.mult,
                    )
                return run

            def wo_group(qb):
                def run():
                    po = [psp.tile([P, FQ], f32, name=f"po{n}_{qb}", tag=f"bg{n}", bufs=1)
                          for n in range(2)]
                    for n in range(2):
                        for j in range(NDB):
                            nc.tensor.matmul(
                                po[n][:, :],
                                lhsT=at_sb[j][:, qb * P:(qb + 1) * P],
                                rhs=wo_sb[:, j, n * FQ:(n + 1) * FQ],
                                start=(j == 0), stop=(j == NDB - 1),
                            )
                    ob = wpool.tile([P, H], f32, name=f"ob_{qb}", tag="ob", bufs=3)
                    for n in range(2):
                        nc.vector.tensor_copy(ob[:, n * FQ:(n + 1) * FQ], po[n][:, :])
                    nc.sync.dma_start(outd[qb * P:(qb + 1) * P, :], ob[:, :])
                return run

            # ---- prologue: K then Q for pair 0 (V streams inside j0/qt0) ----
            for t in range(NQT):
                qkt_group(0, t, "k")()
            for t in range(NQT):
                qkt_group(0, t, "q")()

            background = []

            # ---- attention main loop ----
            for j in range(NDB):
                if j + 1 < NDB:
                    for t in range(NQT):
                        background.append(qkt_group(j + 1, t, "k"))
                        background.append(qkt_group(j + 1, t, "q"))
                for qt in range(NQT):
                    first = (j == 0 and qt == 0)
                    if j == NDB - 1 and qt > 0:
                        for qb in range((qt - 1) * (FQ // P), qt * (FQ // P)):
                            background.append(wo_group(qb))
                    av = [psp.tile([HD + 1, FQ], f32, name=f"av{hh}_{j}_{qt}",
                                   tag=f"av{hh}") for hh in range(2)]
                    es = {}

                    def issue_av(kb):
                        e = es.pop(kb)
                        for hh in range(2):
                            l = 2 * j + hh
                            nc.tensor.matmul(
                                av[hh][:, :],
                                lhsT=v_sb[kb][:, l * (HD + 1):(l + 1) * (HD + 1)],
                                rhs=e[:, hh * FQ:(hh + 1) * FQ],
                                start=(kb == 0), stop=(kb == NKB - 1),
                            )

                    for kb in range(NKB):
                        s = psp.tile([P, 2 * FQ], f32, name=f"s_{j}_{qt}_{kb}",
                                     tag=f"s{kb % 2}", bufs=1)
                        # scores, both heads concurrent via PE row tiling
                        nc.tensor.matmul(
                            s[:, 0:FQ],
                            lhsT=kt_sb[j][0:HD, kb * P:(kb + 1) * P],
                            rhs=qt_sb[j][0:HD, qt * FQ:(qt + 1) * FQ],
                            start=True, stop=True,
                        )
                        nc.tensor.matmul(
                            s[:, FQ:2 * FQ],
                            lhsT=kt_sb[j][HD:P, kb * P:(kb + 1) * P],
                            rhs=qt_sb[j][HD:P, qt * FQ:(qt + 1) * FQ],
                            start=True, stop=True,
                        )
                        e = wpool.tile([P, 2 * FQ], bf16, name=f"e_{j}_{qt}_{kb}",
                                       tag="e", bufs=18)
                        nc.scalar.activation(e[:, :], s[:, :], AF.Exp, scale=float(SCALE))
                        es[kb] = e
                        if kb >= 1:
                            issue_av(kb - 1)
                        if first:
                            v_group(kb)()
                        elif background and (kb % 2 == 1):
                            background.pop(0)()
                    issue_av(NKB - 1)

                    # epilogue: at = (av numerators) / Z, write attn^T rows
                    for hh in range(2):
                        nm = wpool.tile([HD, FQ], f32, name=f"nm_{j}_{qt}_{hh}",
                                        tag="nm", bufs=4)
                        nc.vector.tensor_copy(nm[:, :], av[hh][0:HD, :])
                        # custom DVE op only works with base-partition-0
                        # operands: move Z to partition 0 first (stock copy)
                        zt = wpool.tile([1, FQ], f32, name=f"zt_{j}_{qt}_{hh}",
                                        tag="zt", bufs=2)
                        nc.vector.tensor_copy(zt[0:1, :], av[hh][HD:HD + 1, :])
                        zr32 = wpool.tile([1, FQ], f32, name=f"zr32_{j}_{qt}_{hh}",
                                          tag="zr32", bufs=2)
                        nc.vector.reciprocal_approx_fast(zr32[0:1, :], zt[0:1, :])
                        zrb = wpool.tile([1, FQ], bf16, name=f"zrb_{j}_{qt}_{hh}",
                                         tag="zrb", bufs=2)
                        nc.vector.tensor_copy(zrb[0:1, :], zr32[0:1, :])
                        pbc = psp.tile([HD, FQ], f32, name=f"bc_{j}_{qt}_{hh}",
                                       tag=f"bg{hh}", bufs=1)
                        nc.tensor.matmul(
                            pbc[:, :], lhsT=ones_bf[0:1, 0:HD], rhs=zrb[0:1, :],
                            start=True, stop=True,
                        )
                        nc.vector.tensor_mul(
                            at_sb[j][hh * HD:(hh + 1) * HD, qt * FQ:(qt + 1) * FQ],
                            nm[0:HD, :], pbc[:, :],
                        )
            # remaining background (Wo for q-tile 2) + final q-tile row-blocks
            for fn in background:
                fn()
            for qb in range((NQT - 1) * (FQ // P), NQT * (FQ // P)):
                wo_group(qb)()

    nc.compile()
    return nc


def _shard_inputs(inputs, radial_mask, Wq, bq, Wk, bk, Wv, bv, Wo):
    bf16 = ml_dtypes.bfloat16
    in_maps = []
    for c in range(NCORES):
        b, g = c // G, c % G
        sl = slice(DG * g, DG * (g + 1))
        in_maps.append({
            "xT": np.ascontiguousarray(inputs[b].T).astype(bf16),
            "wq": np.ascontiguousarray(Wq[:, sl]).astype(bf16),
            "wk": np.ascontiguousarray(Wk[:, sl]).astype(bf16),
            "wv": np.ascontiguousarray(Wv[:, sl]).astype(bf16),
            "wo": np.ascontiguousarray(Wo[sl, :]).astype(bf16),
            "bqd": np.ascontiguousarray(bq[sl].reshape(NDB, P).T).astype(np.float32),
            "bkd": np.ascontiguousarray(bk[sl].reshape(NDB, P).T).astype(np.float32),
            "bvd": np.ascontiguousarray(bv[sl].reshape(1, DG)).astype(bf16),
            "maskd": np.ascontiguousarray(radial_mask[b].reshape(NKB, P).T).astype(np.float32),
        })
    return in_maps


def kernel(**inputs):
    from concourse.bass_utils import run_bass_kernel_spmd

    if "nc" not in _CACHE:
        _CACHE["nc"] = _build()
    nc = _CACHE["nc"]

    x = np.asarray(inputs["inputs"], np.float32)
    in_maps = _shard_inputs(
        x, np.asarray(inputs["radial_mask"], np.float32),
        np.asarray(inputs["Wq"], np.float32), np.asarray(inputs["bq"], np.float32),
        np.asarray(inputs["Wk"], np.float32), np.asarray(inputs["bk"], np.float32),
        np.asarray(inputs["Wv"], np.float32), np.asarray(inputs["bv"], np.float32),
        np.asarray(inputs["Wo"], np.float32),
    )

    trace = bool(int(os.environ.get("KERNEL_TRACE", "0")))
    res = run_bass_kernel_spmd(nc, in_maps, core_ids=list(range(NCORES)), trace=trace)
    _CACHE["last_result"] = res

    bo = np.asarray(inputs["bo"], np.float32)
    out = np.empty((B, S, H), np.float32)
    for b in range(B):
        out[b] = res.results[G * b]["out"] + res.results[G * b + 1]["out"] + x[b] + bo
    return out
